# revision 44
# baseline (speedup 1.0000x reference)
"""AttentionBlock (GroupNorm -> 1x1 qkv -> 4-head attention -> 1x1 proj -> residual)
on 8 trn2 NeuronCores, data-parallel over the batch dim (B=8, one element/core).

fp8e4 DoubleRow matmuls for every contraction >= 256 (qkv, V, PV, rowsum,
proj pair0) at ~1.5x PE throughput; ST (contraction=128) stays bf16 and
proj heads 2/3 are fp8-normal singles so only one cheap single gates on the
last head's softmax at the tail. The softmax rowsum is fp8 ones-matmuls on
the PE (a DVE tree loses its 2x mode at 1 byte), so there is no reduction
tree at all; divide = reciprocal + gpsimd partition_broadcast + DVE multiply.

Layout per core: channel-major [C=512, N=1024]. GroupNorm writes h as fp8
"pair" tiles [128, 2, N] (k = channel-tile pair member) feeding DoubleRow
directly. q/k are bf16 channel-major for ST; pt = exp(scale*ST - S0) is
written fp8 into jt-pair tiles [128, 2, N]; v is cast fp8 into jt-pair tiles
[128, 2, 512]; attn = PV*recip(rowsum) is written fp8 into head-pair tiles.

Scaling (exact cancellations; fp8 range/subnormal management):
  q/k weights and biases xWS=32 (kills weight subnormals in e4m3); the exp
    scale divides by WS^2. v weights xVS=16 -> attn carries xVS; proj weights
    xPS=16; the output store multiplies PSUM by 1/(VS*PS), bias added after.
  pt = exp(scale*ST - S0): S0=1.5 keeps max pt ~110 < 240 (TRN e4m3 max, NOT
    the OCP 448); e^-S0 cancels between PV and rowsum. Measured rel err 9.8e-3
    vs the f32 reference (budget 2e-2).

Schedule: PE power-state (K=4/8 array throttle + p-state) is held up by junk
matmul chains through the GroupNorm stretch; q0/k0 accumulate per h-pair
(wave A); remaining qkv + V run under the head-0 exp stream; per head,
rowsum + PV interleave with the next head's ST/exp; proj pair0+h2+bias are
emitted before PV3 to fill PE wait-slack during the final exps. Outputs
store per-half on alternating DVE-stt/ACT paths and stream out over the
SWDGE queue plus both HWDGE queues.
"""

import numpy as np

B, C, H, W = 8, 512, 32, 32
N = H * W  # 1024
NUM_HEADS = 4
HEAD_DIM = C // NUM_HEADS  # 128
NUM_GROUPS = 32
GROUP_CH = C // NUM_GROUPS  # 16
EPS = 1e-5
NT = C // 128  # 4 channel tiles
NO_QK = 8  # q,k output tiles (1024 channels)
N_CORES = 8

WS = 32.0   # qk weight scale (cancelled in exp scale)
VS = 16.0   # v weight scale
PS = 16.0   # proj weight scale
S0 = 1.5    # exp shift
SCALE = 1.0 / (float(np.sqrt(HEAD_DIM)) * WS * WS)
OSC = 1.0 / (VS * PS)


def build_bass():
    import concourse.bacc as bacc
    import concourse.tile as tile
    from concourse import mybir

    f32 = mybir.dt.float32
    bf16 = mybir.dt.bfloat16
    f8 = mybir.dt.float8e4
    Act = mybir.ActivationFunctionType
    Alu = mybir.AluOpType
    Ax = mybir.AxisListType
    DR = mybir.MatmulPerfMode.DoubleRow

    nc = bacc.Bacc("TRN2", target_bir_lowering=False, debug=False,
                   num_devices=N_CORES)

    d_xb = nc.declare_dram_parameter("xb", [C, N], bf16, isOutput=False)
    d_wqkw = nc.declare_dram_parameter("qkw_wt", [128, 1024], f8,
                                       isOutput=False)
    d_wqkr = nc.declare_dram_parameter("qkrest_wt", [128, 3072], f8,
                                       isOutput=False)
    d_wv = nc.declare_dram_parameter("v_wt", [128, 2048], f8, isOutput=False)
    d_pwt = nc.declare_dram_parameter("proj_wt", [128, 2048], f8,
                                      isOutput=False)
    d_cv = nc.declare_dram_parameter("cvec", [128, 24], f32, isOutput=False)
    d_ss = nc.declare_dram_parameter("selsel", [128, 128], bf16,
                                     isOutput=False)
    d_ones8 = nc.declare_dram_parameter("ones8", [128, 32], f8,
                                        isOutput=False)
    d_beffrow = nc.declare_dram_parameter("beffrow", [1, 512], bf16,
                                          isOutput=False)
    d_out = nc.declare_dram_parameter("out", [C, N], bf16, isOutput=True)

    with tile.TileContext(nc) as tc:
        with (
            tc.tile_pool(name="persist", bufs=1) as pp,
            # one slot per exp pair tile for the whole kernel (4 pairs x 4
            # heads): PV consumption never gates a late exp
            tc.tile_pool(name="pt", bufs=16) as p_pt,
            tc.tile_pool(name="outp", bufs=4) as p_out,
            tc.tile_pool(name="small", bufs=1) as ps,
            tc.tile_pool(name="psum", bufs=2, space="PSUM") as pm,
        ):
            # zero operands: the junk streams keep the PE pipeline (p-state /
            # K=8 array power state) up at minimal switching power
            warm = ps.tile([128, 512], bf16, tag="warm", name="warm")
            nc.vector.memset(warm[:], 0.0)
            epsv = ps.tile([128, 1], f32, tag="epsv", name="epsv")
            nc.vector.memset(epsv[:], EPS)
            # pre-load the sqrt ACT table during the DMA dead zone so the
            # first GroupNorm rstd doesn't eat the 1.3us load
            dums = ps.tile([128, 1], f32, tag="dums", name="dums")
            nc.scalar.activation(dums[:], epsv[:, 0:1], Act.Sqrt)

            cvec = ps.tile([128, 24], f32, tag="cvec", name="cvec")
            gam, bet, bqk, beff = (cvec[:, 0:4], cvec[:, 4:8],
                                   cvec[:, 8:16], cvec[:, 16:20])
            ns0 = cvec[:, 20:21]  # -S0 exp bias
            oscv = ps.tile([128, 1], f32, tag="oscv", name="oscv")
            nc.vector.memset(oscv[:], OSC)
            ones_row = ps.tile([1, 512], bf16, tag="ones_row", name="ones_row")
            nc.vector.memset(ones_row[:], 1.0)
            beffrow = ps.tile([1, 512], bf16, tag="beffrow", name="beffrow")
            selsel = ps.tile([128, 128], bf16, tag="selsel", name="selsel")
            ones8 = ps.tile([128, 2, 16], f8, tag="ones8", name="ones8")
            nc.scalar.dma_start(ones8[:], d_ones8[:, :])

            # ---- bulk loads. SWDGE carries the GN->ST critical path in
            # dependency order; the slow HWDGE queues take the small consts
            # and proj weights (needed late) and the outputs.
            xbs, hps = [], []
            for t in range(NT):
                xb_t = pp.tile([128, N], bf16, tag=f"xb{t}", name=f"xb{t}")
                xbs.append(xb_t)
            for p in range(2):
                hp_t = pp.tile([128, 2, N], f8, tag=f"hp{p}", name=f"hp{p}")
                hps.append(hp_t)
            wqkw = pp.tile([128, 2, 512], f8, tag="wqkw", name="wqkw")
            wrs = {}
            for ot in (1, 5, 2, 6, 3, 7):
                wrs[ot] = pp.tile([128, 2, 256], f8, tag=f"wr{ot}",
                                  name=f"wr{ot}")
            wvp = []
            for p in range(2):
                wvp.append(pp.tile([128, 2, 512], f8, tag=f"wv{p}",
                                   name=f"wv{p}"))
            pws = []
            for ot in range(NT):
                pws.append(pp.tile([128, 2, 256], f8, tag=f"pw{ot}",
                                   name=f"pw{ot}"))

            # xb0 split across SWDGE + HWDGE so the first GN reduce starts
            # ~2us earlier; everything downstream shifts left with it
            nc.gpsimd.dma_start(xbs[0][:, 0:512], d_xb[0:128, 0:512])
            nc.sync.dma_start(xbs[0][:, 512:1024], d_xb[0:128, 512:1024])
            nc.gpsimd.dma_start(xbs[1][:], d_xb[128:256, :])
            nc.gpsimd.dma_start(wqkw[:], d_wqkw[:, :])
            nc.gpsimd.dma_start(xbs[2][:], d_xb[256:384, :])
            nc.gpsimd.dma_start(xbs[3][:], d_xb[384:512, :])
            nc.sync.dma_start(selsel[:], d_ss[:, :])
            nc.sync.dma_start(cvec[:], d_cv[:, :])
            nc.sync.dma_start(beffrow[:], d_beffrow[:, :])
            for i, ot in enumerate((1, 5, 2, 6, 3, 7)):
                nc.gpsimd.dma_start(wrs[ot][:], d_wqkr[:, i * 512:(i + 1) * 512])
            for p in range(2):
                nc.gpsimd.dma_start(wvp[p][:], d_wv[:, p * 1024:(p + 1) * 1024])
            for ot in range(NT):
                eng = nc.sync if ot < 2 else nc.scalar
                eng.dma_start(pws[ot][:], d_pwt[:, ot * 512:(ot + 1) * 512])

            # PE warm-up: junk matmul chain (never read) holds the PE p-state
            # up while the first xb tiles stream in.
            junk = pm.tile([128, N], f32, tag="acc", name="junk")

            def junk_mm(n, first=False, last=False):
                for j in range(n):
                    nc.tensor.matmul(junk[0:128, 0:512], warm[:, 0:128],
                                     warm[:, 0:512],
                                     start=(first and j == 0),
                                     stop=(last and j == n - 1),
                                     skip_group_check=True)

            junk_mm(8, first=True, last=True)

            # ---- group norm per-tile (groups never cross 128-ch tiles),
            # h written fp8 into pair tiles; head-0 qkv matmuls ride each
            # completed pair (wave A). Junk matmul bursts keep the PE busy
            # through the GN stretch so the array stays at full power state
            # (K=8/8) and warm p-state for wave A / ST0.
            pqq = pqk = None
            msrs = []
            sqs = ps.tile([128, N], bf16, tag="sqs", name="sqs")
            for t in range(NT):
                # bf16 stats: DVE reduce runs in 2x mode, selsel matmul gets
                # bf16 FWL weight loads; the 0.4% rounding is far below the
                # fp8 h quantization error
                st_t = ps.tile([128, 2], bf16, tag=f"st{t}", name=f"st{t}")
                with nc.allow_low_precision(reason="GN stats 0.4% << fp8 h"):
                    nc.vector.reduce_sum(st_t[:, 0:1], xbs[t][:], axis=Ax.X)
                    nc.scalar.activation(sqs[:], xbs[t][:], Act.Square,
                                         accum_out=st_t[:, 1:2])
                # one fused matmul broadcasts the group (sum, sumsq)/n
                # to channels: selsel = sel @ sel.T / n (block-diagonal,
                # host-precomputed), so no PSUM->SBUF hop between the group
                # reduce and the broadcast.
                pse = pm.tile([128, N], f32, tag="ps", name=f"pse{t}")
                nc.tensor.matmul(pse[:, 0:2], selsel[:], st_t[:, 0:2],
                                 start=True, stop=True)
                if t == 2:
                    # bridge the t2->t3 stats window so the PE never idles
                    # long enough to drop to the K=4/8 array power state
                    # (sized to finish right as pse3/wave-A p1 become ready)
                    junkp = pm.tile([128, N], f32, tag="ps", name="junkp")
                    for j in range(5):
                        nc.tensor.matmul(junkp[0:128, 0:512], warm[:, 0:128],
                                         warm[:, 0:512],
                                         start=(j == 0), stop=(j == 4),
                                         skip_group_check=True)
                if t == 0:
                    # keep PE streaming until wave A takes over (junk's acc
                    # slot is recycled by pqk below, so junk must stop here)
                    junk_mm(5, first=True, last=True)
                    pqq = pm.tile([128, N], f32, tag="acc", name="pqq")
                    pqk = pm.tile([128, N], f32, tag="acc", name="pqk")
                msr = ps.tile([128, 8], f32, tag=f"msr{t}", name=f"msr{t}")
                msrs.append(msr)
                nc.vector.tensor_copy(msr[:, 6:8], pse[:, 0:2])
                # negvar = mean^2 - msq ; rstd = 1/sqrt(eps - negvar)
                nc.vector.scalar_tensor_tensor(msr[:, 0:1], msr[:, 6:7],
                                               msr[:, 6:7], msr[:, 7:8],
                                               op0=Alu.mult, op1=Alu.subtract)
                nc.scalar.activation(msr[:, 1:2], msr[:, 0:1], Act.Sqrt,
                                     bias=epsv[:, 0:1], scale=-1.0)
                nc.vector.reciprocal(msr[:, 2:3], msr[:, 1:2])
                nc.vector.tensor_mul(msr[:, 3:4], gam[:, t:t + 1],
                                     msr[:, 2:3])
                nc.vector.tensor_mul(msr[:, 4:5], msr[:, 6:7], msr[:, 3:4])
                nc.vector.tensor_sub(msr[:, 5:6], bet[:, t:t + 1],
                                     msr[:, 4:5])
                # all h writes on DVE: an ACT h write would sit in front of
                # the GN sqrts in the ACT FIFO and stall every stats chain
                nc.vector.tensor_scalar(hps[t // 2][:, t % 2, :], xbs[t][:],
                                        msr[:, 3:4], msr[:, 5:6],
                                        op0=Alu.mult, op1=Alu.add)

                # wave A: k0 then q0 ride each completed fp8 pair (k first:
                # its bias copy runs on ACT and can overlap q's DVE copy)
                if t % 2 == 1:
                    p = t // 2
                    for pq, wc in ((pqk, 1), (pqq, 0)):
                        for half in range(2):
                            nc.tensor.matmul(
                                pq[:, half * 512:(half + 1) * 512],
                                wqkw[:, :, wc * 256 + p * 128:
                                     wc * 256 + (p + 1) * 128],
                                hps[p][:, :, half * 512:(half + 1) * 512],
                                start=(p == 0), stop=(p == 1),
                                perf_mode=DR)

            # dummy exp reading t3's sqrt output: forces the exp-table load
            # to happen right after the last GN sqrt, off the critical path
            dum = ps.tile([128, 1], f32, tag="dum", name="dum")
            nc.scalar.activation(dum[:], msrs[3][:, 1:2], Act.Exp)

            # bridge the wave-A -> ST0 bias-copy wait (~2.5us) so the PE
            # pipeline stays hot into the ST/exp stream
            junkq = pm.tile([128, N], f32, tag="ps", name="junkq")
            for j in range(4):
                nc.tensor.matmul(junkq[0:128, 0:512], warm[:, 0:128],
                                 warm[:, 0:512], start=(j == 0),
                                 stop=(j == 3), skip_group_check=True)

            qks = [None] * NO_QK
            vps = []
            for m in range(4):
                vps.append(pp.tile([128, 2, 512], f8, tag=f"v{m}",
                                   name=f"v{m}"))
            all_pts = [[None] * 4 for _ in range(NUM_HEADS)]
            attnp = []
            for p in range(2):
                attnp.append(pp.tile([128, 2, N], f8, tag=f"attn{p}",
                                     name=f"attn{p}"))

            def emit_bias(ot, pq, split=False):
                # q biases on DVE, k biases on ACT: the DVE queue otherwise
                # delays the k biases and starves the downstream ST matmuls.
                # split=True stores per half so ST0's first matmuls can start
                # as soon as half 0 lands.
                qk_t = pp.tile([128, N], bf16, tag=f"qk{ot}", name=f"qk{ot}")
                sls = ([slice(0, 512), slice(512, 1024)] if split
                       else [slice(0, N)])
                for sl in sls:
                    if ot == NUM_HEADS:
                        nc.scalar.activation(qk_t[:, sl], pq[:, sl],
                                             Act.Identity,
                                             bias=bqk[:, ot:ot + 1])
                    else:
                        nc.vector.tensor_scalar_add(qk_t[:, sl], pq[:, sl],
                                                    bqk[:, ot:ot + 1])
                qks[ot] = qk_t

            def emit_qkv(ot):
                pq = pm.tile([128, N], f32, tag="acc", name=f"pq{ot}")
                for p in range(2):
                    for half in range(2):
                        nc.tensor.matmul(
                            pq[:, half * 512:(half + 1) * 512],
                            wrs[ot][:, :, p * 128:(p + 1) * 128],
                            hps[p][:, :, half * 512:(half + 1) * 512],
                            start=(p == 0), stop=(p == 1), perf_mode=DR)
                emit_bias(ot, pq)

            def emit_v(jt):
                pv_ = pm.tile([128, N], f32, tag="acc", name=f"pvv{jt}")
                for p in range(2):
                    nc.tensor.matmul(
                        pv_[:, 0:512],
                        hps[p][:, :, jt * 128:(jt + 1) * 128],
                        wvp[p][:, :, :],
                        start=(p == 0), stop=(p == 1), perf_mode=DR)
                nc.vector.tensor_copy(vps[jt // 2][:, jt % 2, :],
                                      pv_[:, 0:512])

            def emit_st(h, jts):
                qT = qks[h]
                kT = qks[NUM_HEADS + h]
                for jt in jts:
                    pst = pm.tile([128, N], f32, tag="ps", name=f"pst{h}_{jt}")
                    for half in range(2):
                        nc.tensor.matmul(
                            pst[:, half * 512:(half + 1) * 512],
                            kT[:, jt * 128:(jt + 1) * 128],
                            qT[:, half * 512:(half + 1) * 512],
                            start=True, stop=True)
                    m = jt // 2
                    if all_pts[h][m] is None:
                        all_pts[h][m] = p_pt.tile([128, 2, N], f8, tag="pt",
                                                  name=f"pt{h}_{m}")
                    nc.scalar.activation(all_pts[h][m][:, jt % 2, :], pst[:],
                                         Act.Exp, bias=ns0, scale=SCALE)

            def emit_rowsum_mm(prs, hp, m, start, stop):
                for half in range(2):
                    sl = slice(half * 512, (half + 1) * 512)
                    nc.tensor.matmul(
                        prs[0:1, sl],
                        ones8[:, :, 0:1],
                        all_pts[hp][m][:, :, sl],
                        start=start, stop=stop, perf_mode=DR)

            def emit_recip(prs, hp):
                rr = ps.tile([1, N], f32, tag="rr", bufs=2, name=f"rr{hp}")
                rb = ps.tile([128, N], f32, tag="rb", bufs=2, name=f"rb{hp}")
                for hf in range(2):
                    sl = slice(hf * 512, (hf + 1) * 512)
                    nc.vector.reciprocal_approx_fast(rr[:, sl], prs[0:1, sl])
                    nc.gpsimd.partition_broadcast(rb[:, sl], rr[:, sl])
                return rb

            def emit_pv_st(hp, hn):
                # interleave PV (and rowsum) of head hp with ST/exp of head
                # hn at jt granularity: the exp stream never sees a long
                # PE burst that would drain the 2-deep pst ring. The last
                # head does its rowsum up front instead - its divide chain
                # is the tail-critical path and the STs are long done.
                last = hp == NUM_HEADS - 1
                ptag = "ps" if last else "acc"
                ppv = prs = rb = None
                if hp is not None:
                    ppv = pm.tile([128, N], f32, tag=ptag, name=f"ppv{hp}")
                    prs = pm.tile([128, N], f32, tag=ptag, name=f"prs{hp}")
                    if last:
                        for m in range(4):
                            emit_rowsum_mm(prs, hp, m, m == 0, m == 3)
                        rb = emit_recip(prs, hp)
                for m in range(4):
                    if hn is not None:
                        emit_st(hn, [2 * m, 2 * m + 1])
                    if hp is not None:
                        if not last:
                            emit_rowsum_mm(prs, hp, m, m == 0, m == 3)
                        for half in range(2):
                            nc.tensor.matmul(
                                ppv[:, half * 512:(half + 1) * 512],
                                vps[m][:, :, hp * 128:(hp + 1) * 128],
                                all_pts[hp][m][:, :,
                                               half * 512:(half + 1) * 512],
                                start=(m == 0), stop=(m == 3), perf_mode=DR)
                if hp is None:
                    return
                if rb is None:
                    rb = emit_recip(prs, hp)
                # per-half multiplies into the fp8 head-pair tile for proj
                for hf in range(2):
                    sl = slice(hf * 512, (hf + 1) * 512)
                    nc.vector.tensor_mul(attnp[hp // 2][:, hp % 2, sl],
                                         ppv[:, sl], rb[:, sl])

            pprs = [None] * NT

            STT_OTS = (0, 2)  # single-DVE-op stores (PE-side rank-1 bias)

            def emit_proj_front(ot):
                # heads 0,1 as one DoubleRow matmul + head 2 single + bias:
                # everything not gated on attn3, emitted early so it fills
                # PE wait-slack during the last exps
                for half in range(2):
                    sl = slice(half * 512, (half + 1) * 512)
                    nc.tensor.matmul(
                        pprs[ot][:, sl],
                        pws[ot][:, :, 0:128],
                        attnp[0][:, :, sl],
                        start=True, stop=False, perf_mode=DR)
                    nc.tensor.matmul(pprs[ot][:, sl],
                                     pws[ot][:, 0, 128:256],
                                     attnp[1][:, 0, sl],
                                     start=False, stop=False)
                    if ot in STT_OTS:
                        nc.tensor.matmul(
                            pprs[ot][:, sl],
                            beffrow[0:1, ot * 128:(ot + 1) * 128],
                            ones_row[0:1, :],
                            start=False, stop=False)

            def emit_proj_h3(ot):
                # the only attn3-gated piece: one cheap fp8 single per half
                for half in range(2):
                    sl = slice(half * 512, (half + 1) * 512)
                    nc.tensor.matmul(pprs[ot][:, sl],
                                     pws[ot][:, 1, 128:256],
                                     attnp[1][:, 1, sl],
                                     start=False, stop=True)

            def emit_out(ot):
                # per-half stores on alternating engines; the outputs stream
                # out over the idle SWDGE queue (gpsimd) plus both HWDGE
                # queues so no single DMA queue serializes the tail.
                o_t = p_out.tile([128, N], bf16, tag="out", name=f"o{ot}")
                ob = None
                if ot not in STT_OTS:
                    ob = p_out.tile([128, N], bf16, tag="ob", name=f"ob{ot}")
                rows = slice(ot * 128, (ot + 1) * 128)
                for hf in range(2):
                    sl = slice(hf * 512, (hf + 1) * 512)
                    if ot in STT_OTS:
                        nc.vector.scalar_tensor_tensor(o_t[:, sl],
                                                       pprs[ot][:, sl],
                                                       oscv[:, 0:1],
                                                       xbs[ot][:, sl],
                                                       op0=Alu.mult,
                                                       op1=Alu.add)
                    else:
                        nc.scalar.activation(ob[:, sl], pprs[ot][:, sl],
                                             Act.Identity,
                                             bias=beff[:, ot:ot + 1],
                                             scale=OSC)
                        nc.vector.tensor_add(o_t[:, sl], ob[:, sl],
                                             xbs[ot][:, sl])
                    # halves over three queues (SWDGE takes two early ones):
                    # no queue drains more than 3 half-tiles on the tail
                    eng = {(0, 0): nc.sync, (0, 1): nc.gpsimd,
                           (1, 0): nc.scalar, (1, 1): nc.gpsimd,
                           (2, 0): nc.sync, (2, 1): nc.scalar,
                           (3, 0): nc.sync, (3, 1): nc.scalar}[(ot, hf)]
                    eng.dma_start(d_out[rows, sl], o_t[:, sl])

            # interleaved schedule: head 0's q/k came from wave A; remaining
            # qkv pairs + V feed heads as their dependencies resolve.
            emit_bias(4, pqk)
            emit_bias(0, pqq)
            emit_st(0, range(NO_QK))
            emit_qkv(1); emit_qkv(5)
            emit_st(1, range(NO_QK))
            emit_qkv(2); emit_qkv(6)
            emit_qkv(3); emit_qkv(7)
            for jt in range(NO_QK):
                emit_v(jt)
            emit_pv_st(0, 2)
            emit_pv_st(1, 3)
            emit_pv_st(2, None)
            # wave 1 of proj (attn pair 0 + head 2) goes in front of PV3 in
            # the PE queue: it fills the PE wait-slack while the last exps
            # stream, leaving only the cheap h3 singles for the tail
            pprs[0] = pm.tile([128, N], f32, tag="acc", name="ppr0")
            pprs[1] = pm.tile([128, N], f32, tag="acc", name="ppr1")
            emit_proj_front(0)
            emit_proj_front(1)
            emit_pv_st(3, None)
            emit_proj_h3(0)
            emit_proj_h3(1)
            emit_out(0)
            emit_out(1)
            pprs[2] = pm.tile([128, N], f32, tag="ps", name="ppr2")
            pprs[3] = pm.tile([128, N], f32, tag="ps", name="ppr3")
            emit_proj_front(2)
            emit_proj_front(3)
            emit_proj_h3(2)
            emit_proj_h3(3)
            emit_out(2)
            emit_out(3)

    nc.compile()
    return nc


def make_in_maps(x, norm_w, norm_b, qkv_w, qkv_b, proj_w, proj_b):
    x = np.asarray(x, dtype=np.float32)
    qkv_w = np.asarray(qkv_w, dtype=np.float32)
    qkv_b = np.asarray(qkv_b, dtype=np.float32)
    proj_w = np.asarray(proj_w, dtype=np.float32)
    proj_b = np.asarray(proj_b, dtype=np.float32)

    import ml_dtypes
    f8 = ml_dtypes.float8_e4m3  # TRN variant: max 240
    wt = np.ascontiguousarray(qkv_w.T)  # [C, 3C] (q 0:512 | k 512:1024 | v)

    A = (wt[:, 0:1024] * WS).reshape(2, 2, 128, 1024)  # [p, k, c_lo, ocol]
    # wave tile [c_lo, k, wc*256 + p*128 + c], wc 0 = q head0, 1 = k head0
    wqkw = np.zeros((128, 2, 2, 2, 128), np.float32)
    for wc, ot in ((0, 0), (1, 4)):
        for p in range(2):
            for k in range(2):
                wqkw[:, k, wc, p, :] = A[p, k, :, ot * 128:(ot + 1) * 128]
    wqkw = wqkw.reshape(128, 1024).astype(f8)
    # rest tiles [c_lo, k, p*128 + c] per ot, in stream order 1,5,2,6,3,7
    wqkr = np.zeros((128, 6, 2, 2, 128), np.float32)
    for i, ot in enumerate((1, 5, 2, 6, 3, 7)):
        for p in range(2):
            for k in range(2):
                wqkr[:, i, k, p, :] = A[p, k, :, ot * 128:(ot + 1) * 128]
    wqkr = wqkr.reshape(128, 3072).astype(f8)

    Vw = (wt[:, 1024:1536] * VS).reshape(2, 2, 128, 512)  # [p, k, c_lo, d]
    wv = np.zeros((128, 2, 2, 512), np.float32)  # [c_lo, p, k, d]
    for p in range(2):
        for k in range(2):
            wv[:, p, k, :] = Vw[p, k]
    wv = wv.reshape(128, 2048).astype(f8)

    Pw = (proj_w.T * PS).reshape(2, 2, 128, 512)  # [p, k, d_lo, ocol]
    pwt = np.zeros((128, 4, 2, 2, 128), np.float32)  # [d_lo, ot, k, p, c]
    for ot in range(4):
        for p in range(2):
            for k in range(2):
                pwt[:, ot, k, p, :] = Pw[p, k, :, ot * 128:(ot + 1) * 128]
    pwt = pwt.reshape(128, 2048).astype(f8)

    b_eff = (proj_b + proj_w @ qkv_b[2 * C:3 * C]).astype(np.float32)
    # q,k leave the qkv matmul scaled by WS, so their biases scale too
    bias_qk = np.ascontiguousarray(qkv_b[:2 * C] * WS)

    p_ = np.arange(128)
    sel = (p_[:, None] // GROUP_CH == np.arange(8)[None, :]).astype(np.float32)
    inv_n = 1.0 / float(GROUP_CH * N)
    selsel = np.ascontiguousarray((sel @ sel.T) * inv_n).astype(ml_dtypes.bfloat16)

    xs = x.reshape(B, C, N)
    cvec = np.zeros((128, 24), np.float32)
    cvec[:, 0:4] = np.asarray(norm_w, np.float32).reshape(4, 128).T
    cvec[:, 4:8] = np.asarray(norm_b, np.float32).reshape(4, 128).T
    cvec[:, 8:16] = bias_qk.reshape(8, 128).T
    cvec[:, 16:20] = b_eff.reshape(4, 128).T
    cvec[:, 20] = -S0
    common = {
        "qkw_wt": wqkw, "qkrest_wt": wqkr, "v_wt": wv, "proj_wt": pwt,
        "cvec": cvec, "selsel": selsel,
        "ones8": np.ones((128, 32), f8),
        "beffrow": (b_eff * VS * PS).reshape(1, 512).astype(ml_dtypes.bfloat16),
    }
    return [dict(common,
                 xb=np.ascontiguousarray(xs[i]).astype(ml_dtypes.bfloat16))
            for i in range(B)]


def run(inputs, trace=False, tmpdir=None):
    from concourse.bass_utils import run_bass_kernel_spmd
    nc = build_bass()
    in_maps = make_in_maps(**inputs)
    res = run_bass_kernel_spmd(nc, in_maps, core_ids=list(range(N_CORES)),
                               trace=trace, tmpdir=tmpdir)
    out = np.stack([res.results[i]["out"] for i in range(N_CORES)])
    return out.reshape(B, C, H, W).astype(np.float32), res


def kernel(**inputs):
    out, _ = run(inputs, trace=False)
    return out
